# revision 29
# baseline (speedup 1.0000x reference)
"""Bass/Trainium2 kernel for nn_KernelizedAttentionResBlock (optimized v4).

Sharding: n-token sharded stream phase (each of 8 cores owns a 128-row
slice of n for ALL batches), one AllGather of x^T, m-sharded FFN in
x^T-native layout (no on-device transposes at all).

Math: S=exp(-0.5(K-mu)^2/(sigma^2+1e-8)) is computed as exp(-(s*K+t)^2)
with s=1/sqrt(2(sigma^2+1e-8)), t=-s*mu; s,t depend only on Q and the
small mu/sigma weights, so the host precomputes them exactly in fp32
(like the baseline's weight folds).  The device streams K/V (fp16),
computes x^T = sum_d S*V + Q^T, AllGathers x^T, and runs the FFN with
the LayerNorm folded into the matmul epilogue:
  LN(x)@w1f.T = r_b * (x @ w1f.T) - (r_b*m_b) * rowsum(w1f),
where m, r come from on-device PE ones-matmul reductions of x^T and
rowsum(w1f) is host-precomputed.

Stream engine split per 4-batch block ([A,D,A,D] pattern):
 - Act:   fused (s*K+t)^2 Square for A-batches + one Exp per 2 batches.
 - DVE:   e=s*K+t (tensor_scalar 4x) for D-batches; S*V multiply +
          D-reduction + Q^T residual in one tensor_tensor_reduce.
 - Pool:  sq=e*e for D-batches.
V DMA lags K DMA by one block so the last ttr chain starts before the
final V bytes land.  Activation tables: natural_log_exp_and_others
(ln/exp/square) serves the stream and the r=exp(-0.5 ln(v)) epilogue;
the sigmoid load hides behind the h1 matmuls.
"""
import sys
import os

sys.path.insert(0, "/opt/trn_rl_repo")

import numpy as np

N = 1024          # n_token
B = 32            # batch
D = 1024          # broadcast dim of K/V
M = 4096          # FFN hidden
NCORES = 8
NSL = N // NCORES     # 128 rows of n per core
MSL = M // NCORES     # 512 FFN hidden units per core
MCH = MSL // 128      # 4 chunks of 128
LN_EPS = 1e-5
NB = 4                # batches per K/V DMA block
NBLK = B // NB

_built = {}
last_results = None  # BassKernelResults of the most recent run (for profiling)


def _build_module():
    """Build (once) the SPMD Bass module run on every core."""
    if "nc" in _built:
        return _built["nc"]

    import concourse.bacc as bacc
    import concourse.mybir as mybir
    import concourse.tile as tile

    AF = mybir.ActivationFunctionType
    ALU = mybir.AluOpType
    f32 = mybir.dt.float32
    f16 = mybir.dt.float16

    nc = bacc.Bacc(trn_type="TRN2", num_devices=NCORES)

    Kd = nc.dram_tensor("Ks", (NSL, B, D), f16, kind="ExternalInput")
    Vd = nc.dram_tensor("Vs", (NSL, B, D), f16, kind="ExternalInput")
    Sd = nc.dram_tensor("Ss", (NSL, B), f32, kind="ExternalInput")
    Td = nc.dram_tensor("Ts", (NSL, B), f32, kind="ExternalInput")
    QTs = nc.dram_tensor("QTs", (NSL, B), f32, kind="ExternalInput")
    W1T = nc.dram_tensor("W1T", (N, MSL), f16, kind="ExternalInput")
    B1 = nc.dram_tensor("B1", (128, MCH), f32, kind="ExternalInput")
    W1S = nc.dram_tensor("W1S", (128, MCH), f32, kind="ExternalInput")
    W2T = nc.dram_tensor("W2T", (MSL, N), f16, kind="ExternalInput")

    XTd = nc.dram_tensor("XT", (NSL, B), f32, kind="ExternalOutput")
    HPd = nc.dram_tensor("HP", (N, B), f32, kind="ExternalOutput")

    cc_in = nc.dram_tensor("cc_in", (NSL, B), f16, kind="Internal")
    cc_out = nc.dram_tensor(
        "cc_out", (N, B), f16, kind="Internal", addr_space="Shared"
    )

    with tile.TileContext(nc) as tc:
        with tc.tile_pool(name="const", bufs=1) as cst, \
             tc.tile_pool(name="small", bufs=1) as sm, \
             tc.tile_pool(name="kp", bufs=4) as kp, \
             tc.tile_pool(name="vp", bufs=4) as vp, \
             tc.tile_pool(name="sq", bufs=3) as sqp, \
             tc.tile_pool(name="es", bufs=3) as esp, \
             tc.tile_pool(name="scr", bufs=4) as scr, \
             tc.tile_pool(name="epool", bufs=6) as ep, \
             tc.tile_pool(name="psum", bufs=1, space="PSUM") as ps:

            sS = cst.tile([NSL, B], f32)
            nc.sync.dma_start(sS[:], Sd[:])
            tS = cst.tile([NSL, B], f32)
            nc.sync.dma_start(tS[:], Td[:])
            # Warm the Act table set containing ln+exp+square so neither the
            # stream nor the r=exp(-0.5 ln(v)) epilogue pays a mid-kernel load.
            warm = sm.tile([128, 1], f32)
            nc.vector.memset(warm[:], 1.0)
            nc.scalar.activation(warm[:], warm[:], AF.Derivative_Erf)

            # ---------- Stream K/V ----------
            # x^T[j, b] = sum_d exp(-(s*K+t)^2) * V + Q^T[j, b]
            # V DMA lags K DMA by one block so sq/exp for the final block are
            # done before its V lands.
            xT = sm.tile([NSL, B], f32)
            kts = {}
            vts = {}

            def _load_k(blk):
                b0 = blk * NB
                kt = kp.tile([NSL, NB, D], f16, tag="kt", name=f"kt{blk}")
                nc.sync.dma_start(kt[:], Kd[:, b0:b0 + NB, :])
                kts[blk] = kt

            def _load_v(blk, half=None):
                b0 = blk * NB
                if blk not in vts:
                    vts[blk] = vp.tile([NSL, NB, D], f16, tag="vt",
                                       name=f"vt{blk}")
                vt = vts[blk]
                if half is None:
                    nc.sync.dma_start(vt[:], Vd[:, b0:b0 + NB, :])
                else:
                    h0 = half * (NB // 2)
                    nc.sync.dma_start(vt[:, h0:h0 + NB // 2, :],
                                      Vd[:, b0 + h0:b0 + h0 + NB // 2, :])

            def _square_phase(blk):
                # e = s*K + t per batch (DVE 4x; batch 3 on Pool), then one
                # Derivative_Erf per 2 batches: S' = 2/sqrt(pi) exp(-e^2).
                b0 = blk * NB
                eg = sqp.tile([NSL, NB, D], f16, tag="sq")
                St = esp.tile([NSL, NB, D], f16, tag="st")
                for i in range(NB):
                    b = b0 + i
                    eng = nc.gpsimd if i == 3 else nc.vector
                    eng.tensor_scalar(eg[:, i, :], kts[blk][:, i, :],
                                      sS[:, b:b + 1], tS[:, b:b + 1],
                                      op0=ALU.mult, op1=ALU.add)
                    if i % 2 == 1:
                        nc.scalar.activation(
                            St[:, i - 1:i + 1, :].rearrange("p n d -> p (n d)"),
                            eg[:, i - 1:i + 1, :].rearrange("p n d -> p (n d)"),
                            AF.Derivative_Erf)
                del kts[blk]
                return St

            xT16 = sm.tile([NSL, B], f16)
            AX = mybir.AxisListType

            def _reduce_phase(blk, St):
                b0 = blk * NB
                for i in range(NB):
                    b = b0 + i
                    # sv = S'*V (fp16 2x; batches 2,3 multiply on Pool),
                    # pairwise folds (2x), short reduce, then
                    # x = sqrt(pi)/2 * red + Q^T (undo the DerivErf scale).
                    mul_eng = nc.gpsimd if i >= 2 else nc.vector
                    sv = scr.tile([NSL, D], f16, tag="sv")
                    mul_eng.tensor_tensor(sv[:], St[:, i, :],
                                          vts[blk][:, i, :], op=ALU.mult)
                    f1 = scr.tile([NSL, D // 2], f16, tag="f1")
                    nc.vector.tensor_tensor(f1[:], sv[:, 0:512],
                                            sv[:, 512:1024], op=ALU.add)
                    f2 = scr.tile([NSL, D // 4], f16, tag="f2")
                    nc.vector.tensor_tensor(f2[:], f1[:, 0:256],
                                            f1[:, 256:512], op=ALU.add)
                    red = scr.tile([NSL, 1], f32, tag="red")
                    nc.vector.tensor_reduce(red[:], f2[:], op=ALU.add,
                                            axis=AX.X)
                    nc.vector.tensor_scalar(xT[:, b:b + 1], red[:],
                                            0.8862269254527579,
                                            qts[:, b:b + 1],
                                            op0=ALU.mult, op1=ALU.add)
                del vts[blk]
                if blk % 2 == 1:
                    lo = (blk - 1) * NB
                    hi = (blk + 1) * NB
                    nc.vector.tensor_copy(xT16[:, lo:hi], xT[:, lo:hi])
                    nc.sync.dma_start(cc_in[:, lo:hi], xT16[:, lo:hi])

            # software pipeline: K(blk) ... V(blk) one block behind
            _load_k(0)
            _load_k(1)
            qts = cst.tile([NSL, B], f32)
            nc.sync.dma_start(qts[:], QTs[:])
            _load_v(0)
            sts = {0: _square_phase(0)}
            for blk in range(1, NBLK):
                _load_k(blk + 1) if blk + 1 < NBLK else None
                if blk == NBLK - 1:
                    _load_v(blk, half=0)
                    _load_v(blk, half=1)
                else:
                    _load_v(blk)
                sts[blk] = _square_phase(blk)
                _reduce_phase(blk - 1, sts.pop(blk - 1))
            _reduce_phase(NBLK - 1, sts.pop(NBLK - 1))

            # ---------- AllGather x^T (native layout, no transpose) ----------
            nc.sync.dma_start(XTd[:], xT[:])
            nc.gpsimd.collective_compute(
                "AllGather", ALU.bypass,
                replica_groups=[list(range(NCORES))],
                ins=[cc_in[:]], outs=[cc_out[:]],
            )
            # During the AG: preload the sqrt table for the r computation.
            nc.scalar.activation(warm[:], warm[:], AF.Sqrt)

            # FFN weights (issued after stream DMAs; needed only post-AG)
            w1T = cst.tile([128, NCORES, MSL], f16)
            nc.sync.dma_start(w1T[:], W1T[:].rearrange("(c p) m -> p c m", p=128))
            b1 = cst.tile([128, MCH], f32)
            nc.sync.dma_start(b1[:], B1[:])
            w1s = cst.tile([128, MCH], f32)
            nc.sync.dma_start(w1s[:], W1S[:])
            w2T = cst.tile([128, MCH, N], f16)
            nc.sync.dma_start(w2T[:], W2T[:].rearrange("(mi p) n -> p mi n", p=128))

            ones_h = cst.tile([128, 1], f16)
            nc.vector.memset(ones_h[:], 1.0)
            ones_r = cst.tile([1, 128], f32)
            nc.vector.memset(ones_r[:], 1.0)

            # x^T gathered: [j, c, b] (n = c*128 + j), fp16
            xTh = sm.tile([128, NCORES, B], f16)
            nc.sync.dma_start(xTh[:], cc_out[:].rearrange("(c j) b -> j c b", j=128))

            # LN stats via PE ones-matmul partition reductions (f16 inputs)
            xsq = sm.tile([128, NCORES, B], f16)
            nc.vector.tensor_tensor(
                xsq[:].rearrange("p c b -> p (c b)"),
                xTh[:].rearrange("p c b -> p (c b)"),
                xTh[:].rearrange("p c b -> p (c b)"), op=ALU.mult)
            sum_ps = ps.tile([1, B], f32, tag="sum")
            for c in range(NCORES):
                nc.tensor.matmul(sum_ps[:], ones_h[:], xTh[:, c, :],
                                 start=(c == 0), stop=(c == NCORES - 1))
            ssq_ps = ps.tile([1, B], f32, tag="ssq")
            for c in range(NCORES):
                nc.tensor.matmul(ssq_ps[:], ones_h[:], xsq[:, c, :],
                                 start=(c == 0), stop=(c == NCORES - 1))

            mrow = sm.tile([1, B], f32)
            nc.vector.tensor_scalar_mul(mrow[:], sum_ps[:], 1.0 / N)
            vq = sm.tile([1, B], f32)
            nc.vector.tensor_scalar_mul(vq[:], ssq_ps[:], 1.0 / N)
            m2 = sm.tile([1, B], f32)
            nc.vector.tensor_tensor(m2[:], mrow[:], mrow[:], op=ALU.mult)
            ve = sm.tile([1, B], f32)
            nc.vector.tensor_tensor(ve[:], vq[:], m2[:], op=ALU.subtract)
            vee = sm.tile([1, B], f32)
            nc.vector.tensor_scalar_add(vee[:], ve[:], LN_EPS)
            # r = 1/sqrt(v+eps)  (sqrt table preloaded during the AG)
            vrec = sm.tile([1, B], f32)
            nc.vector.reciprocal(vrec[:], vee[:])
            rrow = sm.tile([1, B], f32)
            nc.scalar.activation(rrow[:], vrec[:], AF.Sqrt)
            rmrow = sm.tile([1, B], f32)
            nc.vector.tensor_tensor(rmrow[:], rrow[:], mrow[:], op=ALU.mult)

            # broadcast r and r*m to 128 partitions via ones outer-product
            rb_ps = ps.tile([128, B], f32, tag="rb")
            nc.tensor.matmul(rb_ps[:], ones_r[:], rrow[:], start=True, stop=True)
            rmb_ps = ps.tile([128, B], f32, tag="rmb")
            nc.tensor.matmul(rmb_ps[:], ones_r[:], rmrow[:], start=True, stop=True)
            R128 = sm.tile([128, B], f32)
            nc.vector.tensor_copy(R128[:], rb_ps[:])
            RM128 = sm.tile([128, B], f32)
            nc.vector.tensor_copy(RM128[:], rmb_ps[:])

            # ---------- m-sharded FFN on raw x^T with LN folded in ----------
            # z_mi = r*(x@w1f.T)_mi - (r*m)*w1s_mi + b1_mi ; g = z*sigmoid(z)
            g1_sb = sm.tile([128, MCH, B], f16)
            h1_pend = []

            def _silu(mi, h1t):
                t1 = sm.tile([128, B], f32, tag=f"t1_{mi % 2}")
                nc.vector.tensor_tensor(t1[:], h1t[:], R128[:], op=ALU.mult)
                wrm = sm.tile([128, B], f32, tag=f"wrm{mi % 2}")
                nc.vector.tensor_scalar_mul(wrm[:], RM128[:], w1s[:, mi:mi + 1])
                zpre = sm.tile([128, B], f32, tag=f"zp{mi % 2}")
                nc.vector.tensor_tensor(zpre[:], t1[:], wrm[:], op=ALU.subtract)
                sg = sm.tile([128, B], f32, tag=f"sg{mi % 2}")
                nc.scalar.activation(sg[:], zpre[:], AF.Sigmoid,
                                     bias=b1[:, mi:mi + 1])
                nc.vector.scalar_tensor_tensor(
                    g1_sb[:, mi, :], zpre[:], b1[:, mi:mi + 1], sg[:],
                    op0=ALU.add, op1=ALU.mult)

            for mi in range(MCH):
                h1t = ps.tile([128, B], f32, tag=f"h1{mi % 2}")
                for c in range(NCORES):
                    nc.tensor.matmul(h1t[:],
                                     w1T[:, c, mi * 128:(mi + 1) * 128],
                                     xTh[:, c, :],
                                     start=(c == 0), stop=(c == NCORES - 1))
                h1_pend.append((mi, h1t))
                if len(h1_pend) == 2:
                    _silu(*h1_pend.pop(0))
            for mi, h1t in h1_pend:
                _silu(mi, h1t)

            hp_sb = sm.tile([128, NCORES, B], f32)
            for jn in range(NCORES):
                hpt = ps.tile([128, B], f32, tag=("rb", "rmb", "hp0", "hp1")[jn % 4])
                for mi in range(MCH):
                    nc.tensor.matmul(hpt[:],
                                     w2T[:, mi, jn * 128:(jn + 1) * 128],
                                     g1_sb[:, mi, :],
                                     start=(mi == 0), stop=(mi == MCH - 1))
                nc.vector.tensor_copy(hp_sb[:, jn, :], hpt[:])
                if jn == 3:
                    nc.sync.dma_start(
                        HPd[:].rearrange("(jn p) b -> p jn b", p=128)[:, 0:4, :],
                        hp_sb[:, 0:4, :])
            nc.sync.dma_start(
                HPd[:].rearrange("(jn p) b -> p jn b", p=128)[:, 4:8, :],
                hp_sb[:, 4:8, :])

    nc.finalize()
    _built["nc"] = nc
    return nc


def kernel(**inputs):
    from concourse.bass_utils import run_bass_kernel_spmd

    global last_results

    Q = np.asarray(inputs["Q"], dtype=np.float32)
    K = np.asarray(inputs["K"], dtype=np.float32)
    V = np.asarray(inputs["V"], dtype=np.float32)
    mu_w = np.asarray(inputs["mu_w"], dtype=np.float32)
    mu_b = np.asarray(inputs["mu_b"], dtype=np.float32)
    sigma_w = np.asarray(inputs["sigma_w"], dtype=np.float32)
    sigma_b = np.asarray(inputs["sigma_b"], dtype=np.float32)
    ffn_w1 = np.asarray(inputs["ffn_w1"], dtype=np.float32)
    ffn_b1 = np.asarray(inputs["ffn_b1"], dtype=np.float32)
    ffn_w2 = np.asarray(inputs["ffn_w2"], dtype=np.float32)
    ffn_b2 = np.asarray(inputs["ffn_b2"], dtype=np.float32)
    ln_ff_g = np.asarray(inputs["ln_ff_g"], dtype=np.float32)
    ln_ff_b = np.asarray(inputs["ln_ff_b"], dtype=np.float32)
    ln_q_g = np.asarray(inputs["ln_q_g"], dtype=np.float32)
    ln_q_b = np.asarray(inputs["ln_q_b"], dtype=np.float32)

    # ---- Host-side exact precompute (fp32, matches reference math) ----
    # q = LN(Q); mu = tanh(q@mu_w.T+mu_b); sigma = q@sigma_w.T+sigma_b
    # s = 1/sqrt(2*(sigma^2+1e-8)); t = -s*mu
    qmu = Q.mean(axis=-1, keepdims=True)
    qvar = Q.var(axis=-1, keepdims=True)
    qn = (Q - qmu) / np.sqrt(qvar + LN_EPS) * ln_q_g + ln_q_b
    mu = np.tanh(qn @ mu_w.T + mu_b)                  # (B, N)
    sg = qn @ sigma_w.T + sigma_b                     # (B, N)
    sfull = 1.0 / np.sqrt(2.0 * (sg * sg + 1e-8))     # (B, N)
    tfull = -sfull * mu

    # FFN folds of ln_ff into w1/b1; row sums for the LN epilogue fold
    w1f = ffn_w1 * ln_ff_g[None, :]
    b1f = ffn_b1 + ffn_w1 @ ln_ff_b
    w1sums = w1f.sum(axis=1)                          # (M,)

    QT = np.ascontiguousarray(Q.T)                    # (N, B)
    ST = np.ascontiguousarray(sfull.T).astype(np.float32)   # (N, B)
    TT = np.ascontiguousarray(tfull.T).astype(np.float32)
    w1T = np.ascontiguousarray(w1f.T).astype(np.float16)    # (N, M)
    w2T = np.ascontiguousarray(ffn_w2.T).astype(np.float16) # (M, N)
    K16 = K.astype(np.float16)
    V16 = V.astype(np.float16)

    nc = _build_module()

    in_maps = []
    for c in range(NCORES):
        jsl = slice(c * NSL, (c + 1) * NSL)
        msl = slice(c * MSL, (c + 1) * MSL)
        in_maps.append({
            "Ks": np.ascontiguousarray(K16[:, jsl, :].transpose(1, 0, 2)),
            "Vs": np.ascontiguousarray(V16[:, jsl, :].transpose(1, 0, 2)),
            "Ss": np.ascontiguousarray(ST[jsl, :]),
            "Ts": np.ascontiguousarray(TT[jsl, :]),
            "QTs": np.ascontiguousarray(QT[jsl, :]),
            "W1T": np.ascontiguousarray(w1T[:, msl]),
            "B1": np.ascontiguousarray(b1f[msl].reshape(MCH, 128).T),
            "W1S": np.ascontiguousarray(w1sums[msl].reshape(MCH, 128).T),
            "W2T": np.ascontiguousarray(w2T[msl, :]),
        })

    trace = os.environ.get("BASS_KERNEL_TRACE", "0") == "1"
    res = run_bass_kernel_spmd(
        nc, in_maps, core_ids=list(range(NCORES)), trace=trace
    )
    last_results = res

    x = np.concatenate([res.results[c]["XT"] for c in range(NCORES)], axis=0).T
    h = np.zeros((N, B), dtype=np.float32)
    for c in range(NCORES):
        h += res.results[c]["HP"]
    out = x + h.T + ffn_b2[None, :]
    return out.astype(np.float32)


# revision 30
# speedup vs baseline: 1.0005x; 1.0005x over previous
"""Bass/Trainium2 kernel for nn_KernelizedAttentionResBlock (optimized v4).

Sharding: n-token sharded stream phase (each of 8 cores owns a 128-row
slice of n for ALL batches), one AllGather of x^T, m-sharded FFN in
x^T-native layout (no on-device transposes at all).

Math: S=exp(-0.5(K-mu)^2/(sigma^2+1e-8)) is computed as exp(-(s*K+t)^2)
with s=1/sqrt(2(sigma^2+1e-8)), t=-s*mu; s,t depend only on Q and the
small mu/sigma weights, so the host precomputes them exactly in fp32
(like the baseline's weight folds).  The device streams K/V (fp16),
computes x^T = sum_d S*V + Q^T, AllGathers x^T, and runs the FFN with
the LayerNorm folded into the matmul epilogue:
  LN(x)@w1f.T = r_b * (x @ w1f.T) - (r_b*m_b) * rowsum(w1f),
where m, r come from on-device PE ones-matmul reductions of x^T and
rowsum(w1f) is host-precomputed.

Stream engine split per 4-batch block ([A,D,A,D] pattern):
 - Act:   fused (s*K+t)^2 Square for A-batches + one Exp per 2 batches.
 - DVE:   e=s*K+t (tensor_scalar 4x) for D-batches; S*V multiply +
          D-reduction + Q^T residual in one tensor_tensor_reduce.
 - Pool:  sq=e*e for D-batches.
V DMA lags K DMA by one block so the last ttr chain starts before the
final V bytes land.  Activation tables: natural_log_exp_and_others
(ln/exp/square) serves the stream and the r=exp(-0.5 ln(v)) epilogue;
the sigmoid load hides behind the h1 matmuls.
"""
import sys
import os

sys.path.insert(0, "/opt/trn_rl_repo")

import numpy as np

N = 1024          # n_token
B = 32            # batch
D = 1024          # broadcast dim of K/V
M = 4096          # FFN hidden
NCORES = 8
NSL = N // NCORES     # 128 rows of n per core
MSL = M // NCORES     # 512 FFN hidden units per core
MCH = MSL // 128      # 4 chunks of 128
LN_EPS = 1e-5
NB = 4                # batches per K/V DMA block
NBLK = B // NB

_built = {}
last_results = None  # BassKernelResults of the most recent run (for profiling)


def _build_module():
    """Build (once) the SPMD Bass module run on every core."""
    if "nc" in _built:
        return _built["nc"]

    import concourse.bacc as bacc
    import concourse.mybir as mybir
    import concourse.tile as tile

    AF = mybir.ActivationFunctionType
    ALU = mybir.AluOpType
    f32 = mybir.dt.float32
    f16 = mybir.dt.float16

    nc = bacc.Bacc(trn_type="TRN2", num_devices=NCORES)

    Kd = nc.dram_tensor("Ks", (NSL, B, D), f16, kind="ExternalInput")
    Vd = nc.dram_tensor("Vs", (NSL, B, D), f16, kind="ExternalInput")
    Sd = nc.dram_tensor("Ss", (NSL, B), f32, kind="ExternalInput")
    Td = nc.dram_tensor("Ts", (NSL, B), f32, kind="ExternalInput")
    QTs = nc.dram_tensor("QTs", (NSL, B), f32, kind="ExternalInput")
    W1T = nc.dram_tensor("W1T", (N, MSL), f16, kind="ExternalInput")
    B1 = nc.dram_tensor("B1", (128, MCH), f32, kind="ExternalInput")
    W1S = nc.dram_tensor("W1S", (128, MCH), f32, kind="ExternalInput")
    W2T = nc.dram_tensor("W2T", (MSL, N), f16, kind="ExternalInput")

    XTd = nc.dram_tensor("XT", (NSL, B), f32, kind="ExternalOutput")
    HPd = nc.dram_tensor("HP", (N, B), f32, kind="ExternalOutput")

    cc_in = nc.dram_tensor("cc_in", (NSL, B), f16, kind="Internal")
    cc_out = nc.dram_tensor(
        "cc_out", (N, B), f16, kind="Internal", addr_space="Shared"
    )

    with tile.TileContext(nc) as tc:
        with tc.tile_pool(name="const", bufs=1) as cst, \
             tc.tile_pool(name="small", bufs=1) as sm, \
             tc.tile_pool(name="kp", bufs=3) as kp, \
             tc.tile_pool(name="vp", bufs=4) as vp, \
             tc.tile_pool(name="sq", bufs=2) as sqp, \
             tc.tile_pool(name="es", bufs=3) as esp, \
             tc.tile_pool(name="scr", bufs=4) as scr, \
             tc.tile_pool(name="epool", bufs=4) as ep, \
             tc.tile_pool(name="psum", bufs=1, space="PSUM") as ps:

            sS = cst.tile([NSL, B], f32)
            nc.sync.dma_start(sS[:], Sd[:])
            tS = cst.tile([NSL, B], f32)
            nc.sync.dma_start(tS[:], Td[:])
            # Warm the Act table set containing ln+exp+square so neither the
            # stream nor the r=exp(-0.5 ln(v)) epilogue pays a mid-kernel load.
            warm = sm.tile([128, 1], f32)
            nc.vector.memset(warm[:], 1.0)
            nc.scalar.activation(warm[:], warm[:], AF.Derivative_Erf)

            # ---------- Stream K/V ----------
            # x^T[j, b] = sum_d exp(-(s*K+t)^2) * V + Q^T[j, b]
            # V DMA lags K DMA by one block so sq/exp for the final block are
            # done before its V lands.
            xT = sm.tile([NSL, B], f32)
            kts = {}
            vts = {}

            def _load_k(blk):
                b0 = blk * NB
                kt = kp.tile([NSL, NB, D], f16, tag="kt", name=f"kt{blk}")
                nc.sync.dma_start(kt[:], Kd[:, b0:b0 + NB, :])
                kts[blk] = kt

            def _load_v(blk, half=None):
                b0 = blk * NB
                if blk not in vts:
                    vts[blk] = vp.tile([NSL, NB, D], f16, tag="vt",
                                       name=f"vt{blk}")
                vt = vts[blk]
                if half is None:
                    nc.sync.dma_start(vt[:], Vd[:, b0:b0 + NB, :])
                else:
                    h0 = half * (NB // 2)
                    nc.sync.dma_start(vt[:, h0:h0 + NB // 2, :],
                                      Vd[:, b0 + h0:b0 + h0 + NB // 2, :])

            def _square_phase(blk):
                # e = s*K + t per batch (DVE 4x; batch 3 on Pool), then one
                # Derivative_Erf per 2 batches: S' = 2/sqrt(pi) exp(-e^2).
                b0 = blk * NB
                eg = sqp.tile([NSL, NB, D], f16, tag="sq")
                St = esp.tile([NSL, NB, D], f16, tag="st")
                for i in range(NB):
                    b = b0 + i
                    eng = nc.gpsimd if i == 3 else nc.vector
                    eng.tensor_scalar(eg[:, i, :], kts[blk][:, i, :],
                                      sS[:, b:b + 1], tS[:, b:b + 1],
                                      op0=ALU.mult, op1=ALU.add)
                    if i % 2 == 1:
                        nc.scalar.activation(
                            St[:, i - 1:i + 1, :].rearrange("p n d -> p (n d)"),
                            eg[:, i - 1:i + 1, :].rearrange("p n d -> p (n d)"),
                            AF.Derivative_Erf)
                del kts[blk]
                return St

            xT16 = sm.tile([NSL, B], f16)
            AX = mybir.AxisListType

            def _reduce_phase(blk, St):
                b0 = blk * NB
                for i in range(NB):
                    b = b0 + i
                    # sv = S'*V (fp16 2x; batches 2,3 multiply on Pool),
                    # pairwise folds (2x), short reduce, then
                    # x = sqrt(pi)/2 * red + Q^T (undo the DerivErf scale).
                    mul_eng = nc.gpsimd if i >= 2 else nc.vector
                    sv = scr.tile([NSL, D], f16, tag="sv")
                    mul_eng.tensor_tensor(sv[:], St[:, i, :],
                                          vts[blk][:, i, :], op=ALU.mult)
                    f1 = scr.tile([NSL, D // 2], f16, tag="f1")
                    nc.vector.tensor_tensor(f1[:], sv[:, 0:512],
                                            sv[:, 512:1024], op=ALU.add)
                    f2 = scr.tile([NSL, D // 4], f16, tag="f2")
                    nc.vector.tensor_tensor(f2[:], f1[:, 0:256],
                                            f1[:, 256:512], op=ALU.add)
                    red = scr.tile([NSL, 1], f32, tag="red")
                    nc.vector.tensor_reduce(red[:], f2[:], op=ALU.add,
                                            axis=AX.X)
                    nc.vector.tensor_scalar(xT[:, b:b + 1], red[:],
                                            0.8862269254527579,
                                            qts[:, b:b + 1],
                                            op0=ALU.mult, op1=ALU.add)
                del vts[blk]
                if blk % 2 == 1:
                    lo = (blk - 1) * NB
                    hi = (blk + 1) * NB
                    nc.vector.tensor_copy(xT16[:, lo:hi], xT[:, lo:hi])
                    nc.sync.dma_start(cc_in[:, lo:hi], xT16[:, lo:hi])

            # software pipeline: K(blk) ... V(blk) one block behind
            _load_k(0)
            _load_k(1)
            qts = cst.tile([NSL, B], f32)
            nc.sync.dma_start(qts[:], QTs[:])
            _load_v(0)
            sts = {0: _square_phase(0)}
            for blk in range(1, NBLK):
                _load_k(blk + 1) if blk + 1 < NBLK else None
                if blk == NBLK - 1:
                    _load_v(blk, half=0)
                    _load_v(blk, half=1)
                else:
                    _load_v(blk)
                sts[blk] = _square_phase(blk)
                _reduce_phase(blk - 1, sts.pop(blk - 1))
            _reduce_phase(NBLK - 1, sts.pop(NBLK - 1))

            # ---------- AllGather x^T (native layout, no transpose) ----------
            nc.sync.dma_start(XTd[:], xT[:])
            nc.gpsimd.collective_compute(
                "AllGather", ALU.bypass,
                replica_groups=[list(range(NCORES))],
                ins=[cc_in[:]], outs=[cc_out[:]],
            )
            # During the AG: preload the sqrt table for the r computation.
            nc.scalar.activation(warm[:], warm[:], AF.Sqrt)

            # FFN weights (issued after stream DMAs; needed only post-AG)
            w1T = cst.tile([128, NCORES, MSL], f16)
            nc.sync.dma_start(w1T[:], W1T[:].rearrange("(c p) m -> p c m", p=128))
            b1 = cst.tile([128, MCH], f32)
            nc.sync.dma_start(b1[:], B1[:])
            w1s = cst.tile([128, MCH], f32)
            nc.sync.dma_start(w1s[:], W1S[:])
            w2T = cst.tile([128, MCH, N], f16)
            nc.sync.dma_start(w2T[:], W2T[:].rearrange("(mi p) n -> p mi n", p=128))

            ones_h = cst.tile([128, 1], f16)
            nc.vector.memset(ones_h[:], 1.0)
            ones_r = cst.tile([1, 128], f32)
            nc.vector.memset(ones_r[:], 1.0)

            # x^T gathered: [j, c, b] (n = c*128 + j), fp16
            xTh = sm.tile([128, NCORES, B], f16)
            nc.sync.dma_start(xTh[:], cc_out[:].rearrange("(c j) b -> j c b", j=128))

            # LN stats via PE ones-matmul partition reductions (f16 inputs)
            xsq = sm.tile([128, NCORES, B], f16)
            nc.vector.tensor_tensor(
                xsq[:].rearrange("p c b -> p (c b)"),
                xTh[:].rearrange("p c b -> p (c b)"),
                xTh[:].rearrange("p c b -> p (c b)"), op=ALU.mult)
            sum_ps = ps.tile([1, B], f32, tag="sum")
            for c in range(NCORES):
                nc.tensor.matmul(sum_ps[:], ones_h[:], xTh[:, c, :],
                                 start=(c == 0), stop=(c == NCORES - 1))
            ssq_ps = ps.tile([1, B], f32, tag="ssq")
            for c in range(NCORES):
                nc.tensor.matmul(ssq_ps[:], ones_h[:], xsq[:, c, :],
                                 start=(c == 0), stop=(c == NCORES - 1))

            mrow = sm.tile([1, B], f32)
            nc.vector.tensor_scalar_mul(mrow[:], sum_ps[:], 1.0 / N)
            vq = sm.tile([1, B], f32)
            nc.vector.tensor_scalar_mul(vq[:], ssq_ps[:], 1.0 / N)
            m2 = sm.tile([1, B], f32)
            nc.vector.tensor_tensor(m2[:], mrow[:], mrow[:], op=ALU.mult)
            ve = sm.tile([1, B], f32)
            nc.vector.tensor_tensor(ve[:], vq[:], m2[:], op=ALU.subtract)
            vee = sm.tile([1, B], f32)
            nc.vector.tensor_scalar_add(vee[:], ve[:], LN_EPS)
            # r = 1/sqrt(v+eps)  (sqrt table preloaded during the AG)
            vrec = sm.tile([1, B], f32)
            nc.vector.reciprocal(vrec[:], vee[:])
            rrow = sm.tile([1, B], f32)
            nc.scalar.activation(rrow[:], vrec[:], AF.Sqrt)
            rmrow = sm.tile([1, B], f32)
            nc.vector.tensor_tensor(rmrow[:], rrow[:], mrow[:], op=ALU.mult)

            # broadcast r and r*m to 128 partitions via ones outer-product
            rb_ps = ps.tile([128, B], f32, tag="rb")
            nc.tensor.matmul(rb_ps[:], ones_r[:], rrow[:], start=True, stop=True)
            rmb_ps = ps.tile([128, B], f32, tag="rmb")
            nc.tensor.matmul(rmb_ps[:], ones_r[:], rmrow[:], start=True, stop=True)
            R128 = sm.tile([128, B], f32)
            nc.vector.tensor_copy(R128[:], rb_ps[:])
            RM128 = sm.tile([128, B], f32)
            nc.vector.tensor_copy(RM128[:], rmb_ps[:])

            # ---------- m-sharded FFN on raw x^T with LN folded in ----------
            # z_mi = r*(x@w1f.T)_mi - (r*m)*w1s_mi + b1_mi ; g = z*sigmoid(z)
            g1_sb = sm.tile([128, MCH, B], f16)
            h1_pend = []

            def _silu(mi, h1t):
                t1 = sm.tile([128, B], f32, tag=f"t1_{mi % 2}")
                nc.vector.tensor_tensor(t1[:], h1t[:], R128[:], op=ALU.mult)
                wrm = sm.tile([128, B], f32, tag=f"wrm{mi % 2}")
                nc.vector.tensor_scalar_mul(wrm[:], RM128[:], w1s[:, mi:mi + 1])
                zpre = sm.tile([128, B], f32, tag=f"zp{mi % 2}")
                nc.vector.tensor_tensor(zpre[:], t1[:], wrm[:], op=ALU.subtract)
                sg = sm.tile([128, B], f32, tag=f"sg{mi % 2}")
                nc.scalar.activation(sg[:], zpre[:], AF.Sigmoid,
                                     bias=b1[:, mi:mi + 1])
                nc.vector.scalar_tensor_tensor(
                    g1_sb[:, mi, :], zpre[:], b1[:, mi:mi + 1], sg[:],
                    op0=ALU.add, op1=ALU.mult)

            for mi in range(MCH):
                h1t = ps.tile([128, B], f32, tag=f"h1{mi % 2}")
                for c in range(NCORES):
                    nc.tensor.matmul(h1t[:],
                                     w1T[:, c, mi * 128:(mi + 1) * 128],
                                     xTh[:, c, :],
                                     start=(c == 0), stop=(c == NCORES - 1))
                h1_pend.append((mi, h1t))
                if len(h1_pend) == 2:
                    _silu(*h1_pend.pop(0))
            for mi, h1t in h1_pend:
                _silu(mi, h1t)

            hp_sb = sm.tile([128, NCORES, B], f32)
            for jn in range(NCORES):
                hpt = ps.tile([128, B], f32, tag=("rb", "rmb", "hp0", "hp1")[jn % 4])
                for mi in range(MCH):
                    nc.tensor.matmul(hpt[:],
                                     w2T[:, mi, jn * 128:(jn + 1) * 128],
                                     g1_sb[:, mi, :],
                                     start=(mi == 0), stop=(mi == MCH - 1))
                nc.vector.tensor_copy(hp_sb[:, jn, :], hpt[:])
                if jn == 3:
                    nc.sync.dma_start(
                        HPd[:].rearrange("(jn p) b -> p jn b", p=128)[:, 0:4, :],
                        hp_sb[:, 0:4, :])
            nc.sync.dma_start(
                HPd[:].rearrange("(jn p) b -> p jn b", p=128)[:, 4:8, :],
                hp_sb[:, 4:8, :])

    nc.finalize()
    _built["nc"] = nc
    return nc


def kernel(**inputs):
    from concourse.bass_utils import run_bass_kernel_spmd

    global last_results

    Q = np.asarray(inputs["Q"], dtype=np.float32)
    K = np.asarray(inputs["K"], dtype=np.float32)
    V = np.asarray(inputs["V"], dtype=np.float32)
    mu_w = np.asarray(inputs["mu_w"], dtype=np.float32)
    mu_b = np.asarray(inputs["mu_b"], dtype=np.float32)
    sigma_w = np.asarray(inputs["sigma_w"], dtype=np.float32)
    sigma_b = np.asarray(inputs["sigma_b"], dtype=np.float32)
    ffn_w1 = np.asarray(inputs["ffn_w1"], dtype=np.float32)
    ffn_b1 = np.asarray(inputs["ffn_b1"], dtype=np.float32)
    ffn_w2 = np.asarray(inputs["ffn_w2"], dtype=np.float32)
    ffn_b2 = np.asarray(inputs["ffn_b2"], dtype=np.float32)
    ln_ff_g = np.asarray(inputs["ln_ff_g"], dtype=np.float32)
    ln_ff_b = np.asarray(inputs["ln_ff_b"], dtype=np.float32)
    ln_q_g = np.asarray(inputs["ln_q_g"], dtype=np.float32)
    ln_q_b = np.asarray(inputs["ln_q_b"], dtype=np.float32)

    # ---- Host-side exact precompute (fp32, matches reference math) ----
    # q = LN(Q); mu = tanh(q@mu_w.T+mu_b); sigma = q@sigma_w.T+sigma_b
    # s = 1/sqrt(2*(sigma^2+1e-8)); t = -s*mu
    qmu = Q.mean(axis=-1, keepdims=True)
    qvar = Q.var(axis=-1, keepdims=True)
    qn = (Q - qmu) / np.sqrt(qvar + LN_EPS) * ln_q_g + ln_q_b
    mu = np.tanh(qn @ mu_w.T + mu_b)                  # (B, N)
    sg = qn @ sigma_w.T + sigma_b                     # (B, N)
    sfull = 1.0 / np.sqrt(2.0 * (sg * sg + 1e-8))     # (B, N)
    tfull = -sfull * mu

    # FFN folds of ln_ff into w1/b1; row sums for the LN epilogue fold
    w1f = ffn_w1 * ln_ff_g[None, :]
    b1f = ffn_b1 + ffn_w1 @ ln_ff_b
    w1sums = w1f.sum(axis=1)                          # (M,)

    QT = np.ascontiguousarray(Q.T)                    # (N, B)
    ST = np.ascontiguousarray(sfull.T).astype(np.float32)   # (N, B)
    TT = np.ascontiguousarray(tfull.T).astype(np.float32)
    w1T = np.ascontiguousarray(w1f.T).astype(np.float16)    # (N, M)
    w2T = np.ascontiguousarray(ffn_w2.T).astype(np.float16) # (M, N)
    K16 = K.astype(np.float16)
    V16 = V.astype(np.float16)

    nc = _build_module()

    in_maps = []
    for c in range(NCORES):
        jsl = slice(c * NSL, (c + 1) * NSL)
        msl = slice(c * MSL, (c + 1) * MSL)
        in_maps.append({
            "Ks": np.ascontiguousarray(K16[:, jsl, :].transpose(1, 0, 2)),
            "Vs": np.ascontiguousarray(V16[:, jsl, :].transpose(1, 0, 2)),
            "Ss": np.ascontiguousarray(ST[jsl, :]),
            "Ts": np.ascontiguousarray(TT[jsl, :]),
            "QTs": np.ascontiguousarray(QT[jsl, :]),
            "W1T": np.ascontiguousarray(w1T[:, msl]),
            "B1": np.ascontiguousarray(b1f[msl].reshape(MCH, 128).T),
            "W1S": np.ascontiguousarray(w1sums[msl].reshape(MCH, 128).T),
            "W2T": np.ascontiguousarray(w2T[msl, :]),
        })

    trace = os.environ.get("BASS_KERNEL_TRACE", "0") == "1"
    res = run_bass_kernel_spmd(
        nc, in_maps, core_ids=list(range(NCORES)), trace=trace
    )
    last_results = res

    x = np.concatenate([res.results[c]["XT"] for c in range(NCORES)], axis=0).T
    h = np.zeros((N, B), dtype=np.float32)
    for c in range(NCORES):
        h += res.results[c]["HP"]
    out = x + h.T + ffn_b2[None, :]
    return out.astype(np.float32)
